# Initial kernel scaffold
#
"""Trainium2 Bass kernel: 3x3 SAME conv (64->128ch) + bias, double-tanh, min over
channels, for x[16,64,224,224] -> y[16,1,224,224].

Strategy
--------
- Data-parallel over batch: 16 images / 8 NeuronCores = 2 images per core.
  Same NEFF on every core, different input shard (no collectives).
- min_c tanh(tanh(v_c)) == tanh(tanh(min_c v_c)) (tanh is monotone), so the
  double tanh is applied only to the per-pixel channel-minimum.
- Conv as implicit GEMM with the *image patch stationary*: for each tile of
  M=112 output pixels (half an image row), accumulate taps into PSUM[112,128]
  with matmuls lhsT=[K, 112 pixels], rhs=[K, 128 oc].  Output channels land on
  the PSUM free dim, so the channel-min is a native free-dim DVE reduction
  (fused bias-add + min via tensor_tensor_reduce).
- K-packing: the strip tile holds the padded image rows twice (partitions
  0:64 = row r, partitions 64:128 = row r+1), so the taps (kh=0,kw) and
  (kh=1,kw) pair into one K=128 matmul; the (kh=2,kw) taps run with the lower
  64 weight rows zeroed.  9 taps -> 6 matmuls, all K=128.
- Host-side prep (cheap numpy): zero-pad x to [.,64,227,226], pre-transpose
  the weights into the six [128,128] rhs tiles, broadcast bias to [128,128].
"""

import os

import numpy as np
import ml_dtypes

import concourse.bass as bass
import concourse.mybir as mybir
import concourse.tile as tile
from concourse import bacc
from concourse.bass_utils import run_bass_kernel_spmd
from concourse.masks import make_identity

N_CORES = 8
B = 16
BPC = B // N_CORES  # images per core
IC, OC = 64, 128
H = W = 224
PW = 226   # padded row width  (col c = image col c-1; cols 0,225 are zero)
PH = 227   # padded rows       (row r = image row r-1; rows 0,225,226 zero)
R = 28     # output rows per strip
NPOS = R + 2  # padded-row positions held per strip (upper half)
M = 112    # output pixels per matmul tile (half a row)
F32 = mybir.dt.float32

# matmul input dtype (fp32 PSUM accumulation either way)
DT = mybir.dt.bfloat16
DT_NP = ml_dtypes.bfloat16

MIN_INIT = 1.0e30  # init value for the running channel min

_CACHE: dict = {}
LAST_RESULT = None  # BassKernelResults of the most recent run (for profiling)


def _emit(nc: bass.Bass, tc: tile.TileContext, y, xp, wv, ws, bm,
          n_img=BPC, n_strips=H // R):
    """Emit the per-core program.

    y  : [n_img, 1, 224, 224] f32   ExternalOutput
    xp : [n_img, 64, 227, 226] DT   padded input
    wv : [3, 128, 128] DT   rhs tiles for the (kh=0 | kh=1) K-pairs, kw=0..2
    ws : [3, 128, 128] DT   rhs tiles for kh=2 (rows 64:128 are zero)
    bm : [128, 128] f32     bias broadcast to all partitions
    """
    with (
        tc.tile_pool(name="consts", bufs=1) as cpool,
        tc.tile_pool(name="strips", bufs=2) as spool,
        tc.tile_pool(name="stage", bufs=4) as stpool,
        tc.tile_pool(name="obuf", bufs=4) as opool,
        tc.tile_pool(name="cpsum", bufs=6, space="PSUM") as cpsum,
        tc.tile_pool(name="tpsum", bufs=2, space="PSUM") as tpsum,
    ):
        # ---- constants ----
        identity = cpool.tile([128, 128], F32)
        make_identity(nc, identity)
        wv_sb = cpool.tile([128, 3, 128], DT)
        nc.sync.dma_start(wv_sb[:], wv.rearrange("t k n -> k t n"))
        ws_sb = cpool.tile([128, 3, 128], DT)
        nc.sync.dma_start(ws_sb[:], ws.rearrange("t k n -> k t n"))
        bias_mat = cpool.tile([128, 128], F32)
        nc.sync.dma_start(bias_mat[:], bm)

        for b in range(n_img):
            stage_t = None  # [128, 112] f32: col = output tile idx, row = pixel
            for s in range(n_strips):
                h0 = s * R
                # ---- load strip: upper = padded rows h0..h0+NPOS-1,
                #      lower = padded rows h0+1..h0+NPOS ----
                ss = spool.tile([128, NPOS, PW], DT, name="ss")
                nc.sync.dma_start(ss[0:64], xp[b, :, h0:h0 + NPOS, :])
                nc.sync.dma_start(ss[64:128], xp[b, :, h0 + 1:h0 + NPOS + 1, :])
                ssf = ss.rearrange("p a c -> p (a c)")

                for i in range(R):           # output row h = h0 + i
                    for half in range(2):    # output cols [w0, w0+112)
                        w0 = half * M
                        c = (h0 + i) * 2 + half  # global output-tile index
                        psum_t = cpsum.tile([M, 128], F32, name="psum_t")
                        for kw in range(3):  # taps kh=0&1 paired, K=128
                            nc.tensor.matmul(
                                psum_t[:],
                                ssf[:, i * PW + kw + w0: i * PW + kw + w0 + M],
                                wv_sb[:, kw],
                                start=(kw == 0), stop=False,
                            )
                        for kw in range(3):  # taps kh=2 (lower rows x 0)
                            nc.tensor.matmul(
                                psum_t[:],
                                ssf[:, (i + 2) * PW + kw + w0:
                                    (i + 2) * PW + kw + w0 + M],
                                ws_sb[:, kw],
                                start=False, stop=(kw == 2),
                            )
                        # fused bias-add + channel-min -> stage column
                        if c % M == 0:
                            stage_t = stpool.tile([128, M], F32, name="stage_t")
                            nc.any.memzero(stage_t[M:128, :])
                        nc.vector.tensor_tensor_reduce(
                            out=psum_t[:],
                            in0=psum_t[:],
                            in1=bias_mat[0:M, :],
                            scale=1.0,
                            scalar=MIN_INIT,
                            op0=mybir.AluOpType.add,
                            op1=mybir.AluOpType.min,
                            accum_out=stage_t[c % M, None, :][:, :, 0],
                        )
                        if c % M == M - 1:
                            # chunk complete: transpose -> tanh -> tanh -> DMA
                            j = c // M
                            tp = tpsum.tile([M, 128], F32, name="tp")
                            nc.tensor.transpose(tp[:], stage_t[:], identity)
                            ob = opool.tile([M, M], F32, name="ob")
                            nc.scalar.activation(
                                ob[:], tp[:, 0:M],
                                mybir.ActivationFunctionType.Tanh)
                            nc.scalar.activation(
                                ob[:], ob[:],
                                mybir.ActivationFunctionType.Tanh)
                            yflat = y[b, 0].rearrange("h w -> (h w)")
                            nc.sync.dma_start(
                                yflat.rearrange("(t p) -> t p", p=M)
                                [j * M:(j + 1) * M, :],
                                ob[:],
                            )


def _build(n_img=BPC, n_strips=H // R, enable_asserts=False):
    nc = bacc.Bacc(
        "TRN2",
        target_bir_lowering=False,
        debug=False,
        enable_asserts=enable_asserts,
        num_devices=N_CORES,
    )
    xp = nc.dram_tensor("xp", [n_img, IC, PH, PW], DT, kind="ExternalInput")
    wv = nc.dram_tensor("wv", [3, 128, 128], DT, kind="ExternalInput")
    ws = nc.dram_tensor("ws", [3, 128, 128], DT, kind="ExternalInput")
    bm = nc.dram_tensor("bias_mat", [128, 128], F32, kind="ExternalInput")
    y = nc.dram_tensor("y", [n_img, 1, H, W], F32, kind="ExternalOutput")
    with tile.TileContext(nc) as tc:
        _emit(nc, tc, y.ap(), xp.ap(), wv.ap(), ws.ap(), bm.ap(),
              n_img=n_img, n_strips=n_strips)
    nc.compile()
    return nc


def prep_inputs(x, weight, bias):
    """Host-side layout prep (numpy only)."""
    x = np.asarray(x, dtype=np.float32)
    weight = np.asarray(weight, dtype=np.float32)
    bias = np.asarray(bias, dtype=np.float32)
    nb = x.shape[0]
    xp = np.zeros((nb, IC, PH, PW), dtype=np.float32)
    xp[:, :, 1:225, 1:225] = x
    xp = xp.astype(DT_NP)
    wv = np.zeros((3, 128, 128), dtype=np.float32)
    ws = np.zeros((3, 128, 128), dtype=np.float32)
    for kw in range(3):
        wv[kw, 0:64] = weight[:, :, 0, kw].T
        wv[kw, 64:128] = weight[:, :, 1, kw].T
        ws[kw, 0:64] = weight[:, :, 2, kw].T
    wv = np.ascontiguousarray(wv.astype(DT_NP))
    ws = np.ascontiguousarray(ws.astype(DT_NP))
    bm = np.ascontiguousarray(
        np.broadcast_to(bias[None, :], (128, 128)).astype(np.float32))
    return xp, wv, ws, bm


def kernel(x, weight, bias):
    global LAST_RESULT
    xp, wv, ws, bm = prep_inputs(x, weight, bias)
    if "nc" not in _CACHE:
        _CACHE["nc"] = _build()
    nc = _CACHE["nc"]
    in_maps = [
        {
            "xp": np.ascontiguousarray(xp[c * BPC:(c + 1) * BPC]),
            "wv": wv,
            "ws": ws,
            "bias_mat": bm,
        }
        for c in range(N_CORES)
    ]
    res = run_bass_kernel_spmd(nc, in_maps, core_ids=list(range(N_CORES)))
    LAST_RESULT = res
    y = np.concatenate([r["y"] for r in res.results], axis=0)
    return y


# revision 9
# speedup vs baseline: 1.1503x; 1.1503x over previous
"""Trainium2 Bass kernel: 3x3 SAME conv (64->128ch) + bias, double-tanh, min over
channels, for x[16,64,224,224] -> y[16,1,224,224].

Strategy
--------
- Data-parallel over batch: 16 images / 8 NeuronCores = 2 images per core.
  Same NEFF on every core, different input shard (no collectives).
- min_c tanh(tanh(v_c)) == tanh(tanh(min_c v_c)) (tanh is monotone), so the
  double tanh is applied only to the per-pixel channel-minimum.
- Conv as implicit GEMM with the *image patch stationary*: tiles of M=128
  consecutive pixels of the padded row-stream (row stride 226; the 2 pad cols
  per row produce garbage outputs that are dropped at extraction), accumulated
  into PSUM[128, 128oc] with matmuls lhsT=[K, 128 px], rhs=[K, 128 oc].
  M=128 keeps NumWeights==128 so the compiler enables Fast Weight Load.
  Output channels land on the PSUM free dim, so the channel-min is a native
  free-dim DVE reduction.
- K-packing, 9 taps -> 5 K=128 matmuls:
    * strip tile SS: partitions 0:64 = padded row r, 64:128 = row r+1
      -> (kh=0,kw) + (kh=1,kw) pairs, kw = 0,1,2          (3 matmuls)
    * strip tile S2: partitions 0:64 = padded row r+2, 64:128 = same row
      shifted one column -> (kh=2,kw=0) + (kh=2,kw=1) pair (1 matmul)
    * (kh=2,kw=2) single with zeroed lower weight rows     (1 matmul)
- Bias is added on DVE (tensor_tensor add over a 4-tile PSUM bank), then
  reduce_min over the oc axis; minima collect in a stage tile that is
  PE-transposed so pixels become the free dim, double-tanh'd on ScalarE, and
  DMA'd to a DRAM scratch in padded-stream order; one strided DRAM->DRAM DMA
  per image extracts the valid 224x224.
- Host-side prep (cheap numpy): zero-pad x to [.,64,228,227] bf16,
  pre-transpose the weights into five [128,128] rhs tiles, tile bias to
  [128,4,128] f32.
"""

import numpy as np
import ml_dtypes

import concourse.bass as bass
import concourse.mybir as mybir
import concourse.tile as tile
from concourse import bacc
from concourse.bass_utils import run_bass_kernel_spmd
from concourse.masks import make_identity

N_CORES = 8
B = 16
BPC = B // N_CORES  # images per core
IC, OC = 64, 128
H = W = 224
PW = 226    # padded row width in the pixel stream
PWX = 227   # xp width (one extra zero col for the column-shifted strip)
PH = 228    # xp rows (row r = image row r-1; rows 0, 225, 226, 227 zero)
R = 28      # output rows per strip
M = 128     # pixels per matmul tile
NPOS = R + 3   # padded-row positions in the main strip (upper half)
NPOS2 = R + 1  # positions in the kh=2 strip
GTOT = H * PW  # padded-stream length per image (50624)
NT = -(-GTOT // M)  # tiles per image (396)
CH = 128    # stage chunk size (tiles per transpose)
F32 = mybir.dt.float32

DT = mybir.dt.bfloat16
DT_NP = ml_dtypes.bfloat16

_CACHE: dict = {}
LAST_RESULT = None  # BassKernelResults of the most recent run (for profiling)


def _strip_of(t):
    """Strip index owning tile t (by its first pixel row)."""
    return min((t * M) // PW // R, H // R - 1)


def _emit(nc: bass.Bass, tc: tile.TileContext, y, xp, wv, ws, bm,
          n_img=BPC, n_strips=H // R, nrep=1):
    """Emit the per-core program.

    y  : [n_img, 1, 224, 224] f32   ExternalOutput
    xp : [n_img, 64, 228, 227] DT   padded input
    wv : [3, 128, 128] DT   rhs tiles for the (kh=0 | kh=1) K-pairs, kw=0..2
    ws : [2, 128, 128] DT   ws[0]: (kh=2,kw=0 | kh=2,kw=1) pair;
                            ws[1]: (kh=2,kw=2) single, rows 64:128 zero
    bm : [128, 4, 128] f32  bias broadcast to partitions and 4 bank slots
    """
    n_tiles = NT if n_strips == H // R else ((n_strips * R * PW) // M)
    with (
        tc.tile_pool(name="consts", bufs=1) as cpool,
        tc.tile_pool(name="strips", bufs=2) as spool,
        tc.tile_pool(name="strips2", bufs=2) as s2pool,
        tc.tile_pool(name="stage", bufs=4) as stpool,
        tc.tile_pool(name="obuf", bufs=4) as opool,
        tc.tile_pool(name="dscratch", bufs=2, space="DRAM") as dpool,
        tc.tile_pool(name="cpsum", bufs=4, space="PSUM") as cpsum,
        tc.tile_pool(name="tpsum", bufs=2, space="PSUM") as tpsum,
    ):
        # ---- constants ----
        identity = cpool.tile([128, 128], F32)
        make_identity(nc, identity)
        wv_sb = cpool.tile([128, 3, 128], DT)
        nc.sync.dma_start(wv_sb[:], wv.rearrange("t k n -> k t n"))
        ws_sb = cpool.tile([128, 2, 128], DT)
        nc.sync.dma_start(ws_sb[:], ws.rearrange("t k n -> k t n"))
        bias_mat = cpool.tile([128, 4, 128], F32)
        nc.sync.dma_start(bias_mat[:], bm)

        for b in [bb for _ in range(nrep) for bb in range(n_img)]:
            ypad = dpool.tile([NT * M], F32, name="ypad")
            stage_t = None
            psum_t = None
            ssf = s2f = None
            cur_strip = -1
            for t in range(n_tiles):
                s = _strip_of(t)
                if s != cur_strip:
                    cur_strip = s
                    h0 = s * R
                    # main strip: upper rows h0..h0+R+2, lower +1 row
                    ss = spool.tile([128, NPOS, PW], DT, name="ss")
                    nc.sync.dma_start(
                        ss[0:64], xp[b, :, h0:h0 + NPOS, 0:PW])
                    nc.sync.dma_start(
                        ss[64:128], xp[b, :, h0 + 1:h0 + NPOS + 1, 0:PW])
                    ssf = ss.rearrange("p a c -> p (a c)")
                    # kh=2 strip: upper rows h0+2..h0+R+2, lower +1 col
                    s2 = s2pool.tile([128, NPOS2, PW], DT, name="s2")
                    nc.sync.dma_start(
                        s2[0:64], xp[b, :, h0 + 2:h0 + 2 + NPOS2, 0:PW])
                    nc.sync.dma_start(
                        s2[64:128], xp[b, :, h0 + 2:h0 + 2 + NPOS2, 1:PW + 1])
                    s2f = s2.rearrange("p a c -> p (a c)")

                g = t * M - (s * R) * PW  # strip-local stream offset
                q = t % 4
                if q == 0:
                    psum_t = cpsum.tile([M, 4, 128], F32, name="psum_t")
                for kw in range(3):      # (kh=0 | kh=1) pairs
                    nc.tensor.matmul(
                        psum_t[:, q], ssf[:, g + kw: g + kw + M], wv_sb[:, kw],
                        start=(kw == 0), stop=False)
                nc.tensor.matmul(        # (kh=2, kw=0 | kw=1) pair
                    psum_t[:, q], s2f[:, g: g + M], ws_sb[:, 0],
                    start=False, stop=False)
                nc.tensor.matmul(        # (kh=2, kw=2) single
                    psum_t[:, q], s2f[:, g + 2: g + 2 + M], ws_sb[:, 1],
                    start=False, stop=True)

                if t % CH == 0:
                    stage_t = stpool.tile([128, CH], F32, name="stage_t")
                if q == 3 or t == n_tiles - 1:
                    nq = q + 1
                    cc = (t - q) % CH
                    pt_flat = psum_t.rearrange("p q n -> p (q n)")
                    nc.vector.tensor_tensor(
                        pt_flat[:, 0:nq * 128],
                        pt_flat[:, 0:nq * 128],
                        bias_mat.rearrange("p q n -> p (q n)")[:, 0:nq * 128],
                        mybir.AluOpType.add)
                    nc.vector.tensor_reduce(
                        out=stage_t[:, cc:cc + nq],
                        in_=psum_t[:, 0:nq],
                        axis=mybir.AxisListType.X,
                        op=mybir.AluOpType.min)
                if t % CH == CH - 1 or t == n_tiles - 1:
                    # chunk done: transpose -> tanh -> tanh -> scratch DMA
                    j = t // CH
                    w = t % CH + 1  # columns written in this chunk
                    tp = tpsum.tile([CH, 128], F32, name="tp")
                    nc.tensor.transpose(tp[0:w, :], stage_t[:, 0:w], identity)
                    ob = opool.tile([CH, 128], F32, name="ob")
                    nc.scalar.activation(
                        ob[0:w, :], tp[0:w, :],
                        mybir.ActivationFunctionType.Tanh)
                    nc.scalar.activation(
                        ob[0:w, :], ob[0:w, :],
                        mybir.ActivationFunctionType.Tanh)
                    nc.sync.dma_start(
                        ypad.rearrange("(t p) -> t p", p=M)[j * CH:j * CH + w],
                        ob[0:w, :])
            # extract valid pixels: drop the 2 pad cols per padded row
            rows_out = (n_tiles * M) // PW  # complete rows (224 when full)
            nc.sync.dma_start(
                y[b, 0, 0:rows_out, :],
                ypad[0:GTOT].rearrange("(h c) -> h c", c=PW)[0:rows_out, 0:W])


def _build(n_img=BPC, n_strips=H // R, enable_asserts=False, nrep=1):
    # num_devices=1: pure data-parallel SPMD, no collectives — each core runs
    # an independent single-device NEFF on its own input shard.
    nc = bacc.Bacc(
        "TRN2",
        target_bir_lowering=False,
        debug=False,
        enable_asserts=enable_asserts,
        num_devices=1,
    )
    xp = nc.dram_tensor("xp", [n_img, IC, PH, PWX], DT, kind="ExternalInput")
    wv = nc.dram_tensor("wv", [3, 128, 128], DT, kind="ExternalInput")
    ws = nc.dram_tensor("ws", [2, 128, 128], DT, kind="ExternalInput")
    bm = nc.dram_tensor("bias_mat", [128, 4, 128], F32, kind="ExternalInput")
    y = nc.dram_tensor("y", [n_img, 1, H, W], F32, kind="ExternalOutput")
    with tile.TileContext(nc) as tc:
        _emit(nc, tc, y.ap(), xp.ap(), wv.ap(), ws.ap(), bm.ap(),
              n_img=n_img, n_strips=n_strips, nrep=nrep)
    nc.compile()
    return nc


def prep_inputs(x, weight, bias):
    """Host-side layout prep (numpy only)."""
    x = np.asarray(x, dtype=np.float32)
    weight = np.asarray(weight, dtype=np.float32)
    bias = np.asarray(bias, dtype=np.float32)
    nb = x.shape[0]
    xp = np.zeros((nb, IC, PH, PWX), dtype=np.float32)
    xp[:, :, 1:225, 1:225] = x
    xp = xp.astype(DT_NP)
    wv = np.zeros((3, 128, 128), dtype=np.float32)
    ws = np.zeros((2, 128, 128), dtype=np.float32)
    for kw in range(3):
        wv[kw, 0:64] = weight[:, :, 0, kw].T
        wv[kw, 64:128] = weight[:, :, 1, kw].T
    ws[0, 0:64] = weight[:, :, 2, 0].T
    ws[0, 64:128] = weight[:, :, 2, 1].T
    ws[1, 0:64] = weight[:, :, 2, 2].T
    wv = np.ascontiguousarray(wv.astype(DT_NP))
    ws = np.ascontiguousarray(ws.astype(DT_NP))
    bm = np.ascontiguousarray(
        np.broadcast_to(bias[None, None, :], (128, 4, 128)).astype(np.float32))
    return xp, wv, ws, bm


def kernel(x, weight, bias):
    global LAST_RESULT
    xp, wv, ws, bm = prep_inputs(x, weight, bias)
    if "nc" not in _CACHE:
        _CACHE["nc"] = _build()
    nc = _CACHE["nc"]
    in_maps = [
        {
            "xp": np.ascontiguousarray(xp[c * BPC:(c + 1) * BPC]),
            "wv": wv,
            "ws": ws,
            "bias_mat": bm,
        }
        for c in range(N_CORES)
    ]
    res = run_bass_kernel_spmd(nc, in_maps, core_ids=list(range(N_CORES)))
    LAST_RESULT = res
    y = np.concatenate([r["y"] for r in res.results], axis=0)
    return y
